# revision 20
# baseline (speedup 1.0000x reference)
"""Trainium2 Bass kernel for nn_FFT_features (conv1x1+BN+ReLU -> channel FFT ->
conv1x1+BN+ReLU -> channel iFFT magnitude -> conv1x1+BN+ReLU).

Key insight: the FFT/iFFT are over a 16-length channel axis, so they are tiny
dense linear maps.  The whole network collapses to a chain of small
channel-GEMMs + pointwise ops:

    y1  = relu(A1 @ x + c1)         A1 [16,3]   (BN1 folded into conv)
    y2  = relu(A2 @ y1 + c2)        A2 [32,16]  (= BN2*w_mid @ DFT, folded)
    zre = Gre @ y2 ; zim = Gim @ y2 Gre/Gim [16,32] (iFFT real/imag)
    mag = sqrt(zre^2 + zim^2)
    out = relu(A3 @ mag + c3)       A3 [64,16]  (BN3 folded)

Sharding: pure data parallel over 8 NeuronCores, each core takes 256 rows of
the flattened (B*H, W) pixel space (262144 pixels).

On-chip layout: channel GEMMs are stacked block-diagonally into the 128x128 PE
array (8x for stage 1, 4x for stage 2/3, 2x+4 row-tiles for stage 4) so the PE
streams full 128-wide outputs.  Pointwise work (bias+relu evictions, squares,
sqrt) is split between the Scalar (ACT) and Vector (DVE) engines.
"""

import os
import sys

for _p in ("/opt/trn_rl_repo", "/root/.axon_site", "/root/.axon_site/_ro/trn_rl_repo"):
    if os.path.isdir(_p) and _p not in sys.path:
        sys.path.append(_p)

import numpy as np
import ml_dtypes

import concourse.bass as bass
import concourse.bacc as bacc
import concourse.mybir as mybir
import concourse.tile as tile
from contextlib import ExitStack

F32 = mybir.dt.float32
BF16 = mybir.dt.bfloat16
F32R = mybir.dt.float32r

EPS = 1e-5
FCH = 16          # f = out_planes // 4
B, C, H, W = 4, 3, 512, 1024
OC = 64
N_CORES = 8
NPIX_CORE = (B * H * W) // N_CORES     # 262144
ROWS_CORE = (B * H) // N_CORES         # 256 rows of W pixels

# ---- kernel geometry ----
GSZ = 2048        # pixels per group within a load-tile
NG = 8            # groups stacked into the partition dim for stage 1
LT_PIX = GSZ * NG  # 16384 pixels per load-tile (one input DMA)
NQ = 4            # quanta (free-dim slices of 512) per load-tile
QN = 512          # matmul free dim

# variant: "bf16" | "f32r" | "f32"
VARIANT = os.environ.get("KERNEL_VARIANT", "bf16")
# how the stage-4 eviction halves are split between ACT and DVE
EV1_ENGINE = os.environ.get("KERNEL_EV1", "act")      # evict1 engine
SQ_FUSE = os.environ.get("KERNEL_SQ_FUSE", "0") == "1"  # use scalar_tensor_tensor pow fusion


def _fold_bn(w, g, b, m, v):
    s = g.astype(np.float64) / np.sqrt(v.astype(np.float64) + EPS)
    return s[:, None] * w.astype(np.float64), b.astype(np.float64) - m.astype(np.float64) * s


def make_host_weights(w_in, g1, b1, m1, v1, w_mid, g2, b2, m2, v2, w_out, g3, b3, m3, v3):
    """Fold BN + DFT/iDFT into 4 small matrices, then lay them out as the
    block-diagonal stacked lhsT tiles + per-partition bias vectors."""
    f = FCH
    A1, c1 = _fold_bn(w_in, g1, b1, m1, v1)            # [16,3]
    k = np.arange(f)
    F = np.exp(-2j * np.pi * np.outer(k, k) / f)
    Fmat = np.concatenate([F.real, F.imag], axis=0)     # [32,16]
    A2w, c2 = _fold_bn(w_mid, g2, b2, m2, v2)           # [32,32]
    A2 = A2w @ Fmat                                     # [32,16]
    co = np.cos(2 * np.pi * np.outer(k, k) / f) / f
    si = np.sin(2 * np.pi * np.outer(k, k) / f) / f
    G_re = np.concatenate([co, -si], axis=1)            # [16,32]
    G_im = np.concatenate([si, co], axis=1)             # [16,32]
    A3, c3 = _fold_bn(w_out, g3, b3, m3, v3)            # [64,16]

    lhsT1 = np.zeros((24, 128), np.float64)
    for g in range(8):
        # rhs partition 3g+c ; out partition 16g+o
        lhsT1[3 * g:3 * g + 3, 16 * g:16 * g + 16] = A1.T
    lhsT2 = np.zeros((128, 128), np.float64)
    for base in (0, 64):
        for gp in range(4):
            lhsT2[base + 16 * gp: base + 16 * gp + 16, 32 * gp:32 * gp + 32] = A2.T
    lhsT3 = np.zeros((128, 128), np.float64)
    for gp in range(4):
        lhsT3[32 * gp:32 * gp + 32, 16 * gp:16 * gp + 16] = G_re.T
        lhsT3[32 * gp:32 * gp + 32, 64 + 16 * gp:64 + 16 * gp + 16] = G_im.T
    lhsT4 = np.zeros((128, 128), np.float64)
    for t in range(4):
        for d in range(2):
            lhsT4[32 * t + 16 * d:32 * t + 16 * d + 16, 64 * d:64 * d + 64] = A3.T

    bias1 = np.tile(c1, 8).astype(np.float32).reshape(128, 1)
    bias2 = np.tile(c2, 4).astype(np.float32).reshape(128, 1)
    bias4 = np.tile(c3, 2).astype(np.float32).reshape(128, 1)
    return dict(lhsT1=lhsT1, lhsT2=lhsT2, lhsT3=lhsT3, lhsT4=lhsT4,
                bias1=bias1, bias2=bias2, bias4=bias4)


def build_nc(n_pix=NPIX_CORE, variant=VARIANT, ev1_engine=EV1_ENGINE, sq_fuse=SQ_FUSE):
    assert n_pix % LT_PIX == 0
    nlt = n_pix // LT_PIX

    if variant == "bf16":
        DT = BF16
    else:
        DT = F32

    def mmv(ap):
        # view an AP with the matmul dtype (f32r runs the PE at 1 col/cycle)
        if variant == "f32r":
            return ap.bitcast(F32R)
        return ap

    nc = bacc.Bacc("TRN2", target_bir_lowering=False, debug=False,
                   num_devices=N_CORES)
    img = nc.dram_tensor("img_slab", [3, n_pix], F32, kind="ExternalInput")
    wt1 = nc.dram_tensor("lhsT1", [24, 128], DT, kind="ExternalInput")
    wt2 = nc.dram_tensor("lhsT2", [128, 128], DT, kind="ExternalInput")
    wt3 = nc.dram_tensor("lhsT3", [128, 128], DT, kind="ExternalInput")
    wt4 = nc.dram_tensor("lhsT4", [128, 128], DT, kind="ExternalInput")
    bs1 = nc.dram_tensor("bias1", [128, 1], F32, kind="ExternalInput")
    bs2 = nc.dram_tensor("bias2", [128, 1], F32, kind="ExternalInput")
    bs4 = nc.dram_tensor("bias4", [128, 1], F32, kind="ExternalInput")
    out = nc.dram_tensor("out_slab", [64, n_pix], F32, kind="ExternalOutput")

    # DRAM views matching the on-chip partition layouts.  DMA matches source
    # and dest in flat AP-iteration order, so a [g,c,n] source view lines up
    # with a [(g c), n] SBUF tile, etc.
    in_view = img.rearrange("c (lt g n) -> lt g c n", lt=nlt, g=NG, n=GSZ)
    # out DMA per (load-tile, d): DRAM side [o, t, (tq j)] — 3 dims with an
    # 8KB contiguous inner run; SBUF side is a [64, 8192] contiguous slab.
    out_view = out.rearrange("o (lt t d n) -> lt d o t n",
                             lt=nlt, t=4, d=2, n=GSZ)

    Relu = mybir.ActivationFunctionType.Relu
    Sqrt = mybir.ActivationFunctionType.Sqrt
    Square = mybir.ActivationFunctionType.Square
    ADD = mybir.AluOpType.add
    MAX = mybir.AluOpType.max
    MULT = mybir.AluOpType.mult
    POW = mybir.AluOpType.pow

    with tile.TileContext(nc) as tc, ExitStack() as ctx:
        wpool = ctx.enter_context(tc.tile_pool(name="weights", bufs=1))
        lpool = ctx.enter_context(tc.tile_pool(name="load", bufs=2))
        y1pool = ctx.enter_context(tc.tile_pool(name="y1", bufs=2))
        y2pool = ctx.enter_context(tc.tile_pool(name="y2", bufs=2))
        sqpool = ctx.enter_context(tc.tile_pool(name="sq", bufs=2))
        magpool = ctx.enter_context(tc.tile_pool(name="mag", bufs=2))
        opool = ctx.enter_context(tc.tile_pool(name="ostage", bufs=2))
        p1pool = ctx.enter_context(tc.tile_pool(name="p1", bufs=2, space="PSUM"))
        p2pool = ctx.enter_context(tc.tile_pool(name="p2", bufs=1, space="PSUM"))
        p3repool = ctx.enter_context(tc.tile_pool(name="p3re", bufs=1, space="PSUM"))
        p3impool = ctx.enter_context(tc.tile_pool(name="p3im", bufs=1, space="PSUM"))
        p4pool = ctx.enter_context(tc.tile_pool(name="p4", bufs=1, space="PSUM"))

        lhsT1_sb = wpool.tile([24, 128], DT)
        nc.sync.dma_start(lhsT1_sb[:], wt1[:])
        lhsT2_sb = wpool.tile([128, 128], DT)
        nc.sync.dma_start(lhsT2_sb[:], wt2[:])
        lhsT3_sb = wpool.tile([128, 128], DT)
        nc.sync.dma_start(lhsT3_sb[:], wt3[:])
        lhsT4_sb = wpool.tile([128, 128], DT)
        nc.sync.dma_start(lhsT4_sb[:], wt4[:])
        bias1_sb = wpool.tile([128, 1], F32)
        nc.sync.dma_start(bias1_sb[:], bs1[:])
        bias2_sb = wpool.tile([128, 1], F32)
        nc.sync.dma_start(bias2_sb[:], bs2[:])
        bias4_sb = wpool.tile([128, 1], F32)
        nc.sync.dma_start(bias4_sb[:], bs4[:])

        def load_lt(i):
            Lt = lpool.tile([24, GSZ], DT, name=f"L{i}", tag="L")
            if variant == "bf16":
                nc.gpsimd.dma_start(Lt[:], in_view[i])   # SWDGE casts f32->bf16
            else:
                nc.sync.dma_start(Lt[:], in_view[i])
            return Lt

        L_next = load_lt(0)
        for lt in range(nlt):
            L = L_next
            if lt + 1 < nlt:
                # prefetch next load-tile before this tile's out-DMAs enter the
                # gpsimd queue, so the queue wait on O doesn't starve stage 1
                L_next = load_lt(lt + 1)
            # per-load-tile output staging slab; free index = 2048*t + 512*tq + j
            O = opool.tile([128, 4 * NQ * QN], F32)
            Q_lt = sqpool.tile([128, NQ * QN], F32, tag="q")
            mag_lt = magpool.tile([128, NQ * QN], DT)
            for tq in range(NQ):
                rhs1 = L[:, tq * QN:(tq + 1) * QN]
                # ---- stage 1: [3 -> 16] x8 groups ----
                P1 = p1pool.tile([128, QN], F32)
                nc.tensor.matmul(P1[:], mmv(lhsT1_sb[:]), mmv(rhs1))
                y1 = y1pool.tile([128, QN], DT)
                if ev1_engine == "act":
                    nc.scalar.activation(y1[:], P1[:], Relu, bias=bias1_sb[:])
                else:
                    nc.vector.tensor_scalar(y1[:], P1[:], bias1_sb[:], 0.0, ADD, MAX)
                # ---- stage 2: [16 -> 32] x4 groups, two halves ----
                P2 = p2pool.tile([128, 2 * QN], F32)
                nc.tensor.matmul(P2[:, 0:QN], mmv(lhsT2_sb[0:64, :]), mmv(y1[0:64, :]))
                nc.tensor.matmul(P2[:, QN:2 * QN], mmv(lhsT2_sb[64:128, :]), mmv(y1[64:128, :]))
                y2 = y2pool.tile([128, 2 * QN], DT)
                nc.vector.tensor_scalar(y2[:], P2[:], bias2_sb[:], 0.0, ADD, MAX)
                # ---- stage 3: iFFT [32 -> 16re + 16im] x4 groups, two halves ----
                P3re = p3repool.tile([128, QN], F32)
                P3im = p3impool.tile([128, QN], F32)
                nc.tensor.matmul(P3re[0:64, :], mmv(lhsT3_sb[:, 0:64]), mmv(y2[:, 0:QN]))
                nc.tensor.matmul(P3re[64:128, :], mmv(lhsT3_sb[:, 0:64]), mmv(y2[:, QN:2 * QN]))
                nc.tensor.matmul(P3im[0:64, :], mmv(lhsT3_sb[:, 64:128]), mmv(y2[:, 0:QN]))
                nc.tensor.matmul(P3im[64:128, :], mmv(lhsT3_sb[:, 64:128]), mmv(y2[:, QN:2 * QN]))
                # ---- magnitude ----
                S1 = sqpool.tile([128, QN], F32, tag="s1")
                nc.scalar.activation(S1[:], P3re[:], Square)
                S2 = sqpool.tile([128, QN], F32, tag="s2")
                nc.scalar.activation(S2[:], P3im[:], Square)
                nc.vector.tensor_tensor(Q_lt[:, tq * QN:(tq + 1) * QN], S1[:], S2[:], ADD)
            # one batched sqrt per load-tile (FD 2048 amortizes overhead)
            nc.scalar.activation(mag_lt[:], Q_lt[:], Sqrt)
            for tq in range(NQ):
                # ---- stage 4: [16 -> 64] x2 groups, 4 row-tiles ----
                # eviction target: O columns {2048*t + 512*tq + j} for each t
                Ov = O[:].rearrange("p (t n) -> p t n", t=4)[:, :, tq * QN:(tq + 1) * QN]
                mg = mag_lt[:, tq * QN:(tq + 1) * QN]
                P4a = p4pool.tile([128, 2 * QN], F32, tag="p4")
                nc.tensor.matmul(P4a[:, 0:QN], mmv(lhsT4_sb[0:32, :]), mmv(mg[0:32, :]),
                                 tile_position=(0, 0))
                nc.tensor.matmul(P4a[:, QN:2 * QN], mmv(lhsT4_sb[32:64, :]), mmv(mg[32:64, :]),
                                 tile_position=(32, 0))
                nc.scalar.activation(Ov[:, 0:2, :], P4a[:], Relu, bias=bias4_sb[:])
                P4b = p4pool.tile([128, 2 * QN], F32, tag="p4")
                nc.tensor.matmul(P4b[:, 0:QN], mmv(lhsT4_sb[64:96, :]), mmv(mg[64:96, :]),
                                 tile_position=(64, 0))
                nc.tensor.matmul(P4b[:, QN:2 * QN], mmv(lhsT4_sb[96:128, :]), mmv(mg[96:128, :]),
                                 tile_position=(96, 0))
                nc.vector.tensor_scalar(Ov[:, 2:4, :], P4b[:], bias4_sb[:], 0.0, ADD, MAX)
            for dd in range(2):
                # 2MB SWDGE DMA; DRAM inner run is 8KB contiguous
                nc.gpsimd.dma_start(out_view[lt, dd], O[64 * dd:64 * dd + 64, :])
    nc.compile()
    return nc


def host_pipeline(img_slab, hw):
    """Numpy model of exactly what the device computes (for sim verification)."""
    x = img_slab.astype(np.float64)                    # [3, n]
    A1 = hw["lhsT1"][0:3, 0:16].T
    y1 = np.maximum(A1 @ x + hw["bias1"][0:16], 0)
    A2 = hw["lhsT2"][0:16, 0:32].T
    y2 = np.maximum(A2 @ y1 + hw["bias2"][0:32], 0)
    Gre = hw["lhsT3"][0:32, 0:16].T
    Gim = hw["lhsT3"][0:32, 64:80].T
    zre = Gre @ y2
    zim = Gim @ y2
    mag = np.sqrt(zre * zre + zim * zim)
    A3 = hw["lhsT4"][0:16, 0:64].T
    y3 = np.maximum(A3 @ mag + hw["bias4"][0:64], 0)
    return y3.astype(np.float32)


_CACHE = {}


def _np_dt(variant):
    return ml_dtypes.bfloat16 if variant == "bf16" else np.float32


def kernel(img, w_in, g1, b1, m1, v1, w_mid, g2, b2, m2, v2, w_out, g3, b3, m3, v3,
           trace=False):
    from concourse.bass_utils import run_bass_kernel_spmd

    variant = VARIANT
    hw = make_host_weights(w_in, g1, b1, m1, v1, w_mid, g2, b2, m2, v2,
                           w_out, g3, b3, m3, v3)
    ndt = _np_dt(variant)
    weight_args = {
        "lhsT1": np.ascontiguousarray(hw["lhsT1"].astype(ndt)),
        "lhsT2": np.ascontiguousarray(hw["lhsT2"].astype(ndt)),
        "lhsT3": np.ascontiguousarray(hw["lhsT3"].astype(ndt)),
        "lhsT4": np.ascontiguousarray(hw["lhsT4"].astype(ndt)),
        "bias1": hw["bias1"], "bias2": hw["bias2"], "bias4": hw["bias4"],
    }

    key = variant
    if key not in _CACHE:
        _CACHE[key] = build_nc(variant=variant)
    nc = _CACHE[key]

    # core i handles rows [256i, 256(i+1)) of the flattened (B*H, W) space
    imgf = np.asarray(img, np.float32).reshape(B, 3, H * W)
    in_maps = []
    for i in range(N_CORES):
        b = (ROWS_CORE * i) // H
        h0 = (ROWS_CORE * i) % H
        slab = np.ascontiguousarray(
            imgf[b, :, h0 * W:(h0 + ROWS_CORE) * W])   # [3, NPIX_CORE]
        in_maps.append({"img_slab": slab, **weight_args})

    res = run_bass_kernel_spmd(nc, in_maps, list(range(N_CORES)), trace=trace)
    kernel.last_results = res

    outp = np.empty((B, OC, H, W), np.float32)
    for i in range(N_CORES):
        b = (ROWS_CORE * i) // H
        h0 = (ROWS_CORE * i) % H
        outp[b, :, h0:h0 + ROWS_CORE, :] = \
            res.results[i]["out_slab"].reshape(OC, ROWS_CORE, W)
    return outp


kernel.last_results = None
